# revision 30
# baseline (speedup 1.0000x reference)
"""Trainium2 Bass kernel for nn_GumbelLinear (topk_masking).

Computation:
  h (64,16) -> conditional range-remap (global min/max of h) ->
  mask = h @ w_p + bias -> logits = mask + g1 - g2 (Gumbel noise from
  U1/U2) -> per-row top-5 hard mask (straight-through).

Sharding: replicate h (needed for the global min/max) and w_p; data-parallel
the 64-row axis across 8 cores (8 rows each).

Key restructure vs the straightforward lowering: the conditional remap is a
global affine map hu = A*h + B (the clip endpoints are exactly attained at
h_min/h_max, so the clip is a mathematical no-op inside the range), hence

  logits = A*(h @ w_p) + B*colsum(w_p) + bias + g1 - g2.

This takes the matmul OFF the min/max critical path: pm = h @ w_p and
ws = colsum(w_p) (a ones-lhsT matmul, which also pre-broadcasts it across
partitions) fire as soon as the input DMA lands, in parallel with the DVE
reduction chain that produces A and B.  The Gumbel branch (ACT engine lns +
GpSimd adds) also runs in parallel; two fused DVE ops combine everything.

Device notes:
  - Split input DMA from two contiguous DRAM tensors: SP triggers the
    DVE/PE-critical columns (hT|hTs|ones|w_p), the Activation engine (the
    other HWDGE trigger source) brings U1|U2|bias in parallel.
  - Global max/min: two X-reduces (max, negated min) into adjacent columns,
    then ONE apply_transpose X-reduce (transposes the 32x32 block in the
    reduce itself) gives gmax on partition 0 / -min on partition 1; the
    following [1,1] scalar chain mixes them via APs with different start
    partitions (lanes align by index, so partition offsets substitute for
    an explicit shuffle).
  - sigmoid is monotonic, so the top-5 threshold compare runs on logits
    directly and the straight-through output is the 0/1 mask itself.
  - A dependency-free dummy Ln anchors the ACT table load before the DMA
    wait so it overlaps the input DMA.
  - _trim_overhead() strips framework pieces that only matter for
    multi-context NEFFs (init barrier, exit barriers/sem clears, the
    set-0 ACT table load) and releases the output-DMA trigger two DVE ops
    early so its ~1.3us pipeline overlaps the compute tail.
"""

import numpy as np

N_CORES = 8
ROWS = 64
D = 16
RPC = ROWS // N_CORES  # rows per core
EPS = 1e-8
NEG = -1.0e30

# packed_sp layout [16, 96] (contiguous)
C_HT = 0      # [0:16,  0:64]  hT (full h, transposed)
C_HTS = 64    # [0:16, 64:72]  this core's 8 rows of h, transposed
C_ONES = 72   # [0:16, 72:80]  ones (colsum lhsT)
C_WP = 80     # [0:16, 80:96]  w_p
CS_END = 96
# packed_act layout [8, 48] (contiguous)
C_U = 0       # [0:8,  0:32]  [U1 | U2] rows (flattened)
C_BIAS = 32   # [0:8, 32:48]  bias rows
CA_END = 48

_CACHE = {}


def _build_nc():
    import concourse.tile as tile
    from concourse import bacc, mybir

    f32 = mybir.dt.float32
    Alu = mybir.AluOpType
    Act = mybir.ActivationFunctionType

    nc = bacc.Bacc("TRN2", debug=False, enable_asserts=False)

    packed_sp = nc.dram_tensor("packed_sp", (16, CS_END), f32, kind="ExternalInput")
    packed_act = nc.dram_tensor("packed_act", (RPC, CA_END), f32, kind="ExternalInput")
    out_s = nc.dram_tensor("out_s", (RPC, D), f32, kind="ExternalOutput")

    with tile.TileContext(nc) as tc:
        with (
            tc.tile_pool(name="sb", bufs=1) as sb,
            tc.tile_pool(name="ps", bufs=1, space=tile.bass.MemorySpace.PSUM) as ps,
        ):
            # split input DMA first: SP brings the DVE/PE-critical columns,
            # the Activation engine brings U/bias in parallel.  Emitted
            # before the dummy Ln so the ACT stream runs [dma, table, lns].
            t = sb.tile([16, CS_END], f32)
            nc.sync.dma_start(t[:], packed_sp[:], single_packet=True)
            ta = sb.tile([RPC, CA_END], f32)
            nc.scalar.dma_start(ta[:], packed_act[:], single_packet=True)

            eps_t = sb.tile([RPC, 1], f32)
            nc.gpsimd.memset(eps_t[:], EPS)
            scr = sb.tile([32, 33], f32)
            nc.vector.memset(scr[:], NEG)
            tb = sb.tile([32, 2], f32)
            nc.vector.memset(tb[:], 0.0)
            # const lane pairs for the fused (X - C)*s + D step
            cc = sb.tile([1, 2], f32)
            nc.vector.memset(cc[0:1, 0:1], 1.0)
            nc.vector.memset(cc[0:1, 1:2], 0.3)
            dd = sb.tile([1, 2], f32)
            nc.vector.memset(dd[0:1, 0:1], 1.0)
            nc.vector.memset(dd[0:1, 1:2], 0.0)
            dscr = sb.tile([1, 1], f32)
            nc.scalar.activation(
                dscr[:], eps_t[0:1, 0:1], Act.Ln, bias=eps_t[0:1, :], scale=1.0
            )

            v_hT = t[0:16, C_HT:C_HTS]     # [16,64]
            v_hTs = t[0:16, C_HTS:C_ONES]  # [16,8]
            v_ones = t[0:16, C_ONES:C_WP]  # [16,8]
            v_wp = t[0:16, C_WP:CS_END]    # [16,16]
            v_u = ta[:, C_U:C_BIAS]        # [8,32]  [u1 | u2]
            v_bias = ta[:, C_BIAS:CA_END]  # [8,16]

            # ---- PE: pm = h @ w_p ; ws = colsum(w_p) broadcast to 8 rows --
            pm = ps.tile([RPC, D], f32)
            nc.tensor.matmul(pm[:], v_hTs, v_wp, start=True, stop=True)
            ws = ps.tile([RPC, D], f32)
            nc.tensor.matmul(ws[:], v_ones, v_wp, start=True, stop=True)

            # ---- ACT: Gumbel b = ln(-ln(U + eps) + eps) for U1|U2 packed --
            a_ = sb.tile([RPC, 32], f32)
            nc.scalar.activation(a_[:], v_u, Act.Ln, bias=eps_t[:], scale=1.0)
            b_ = sb.tile([RPC, 32], f32)
            nc.scalar.activation(b_[:], a_[:], Act.Ln, bias=eps_t[:], scale=-1.0)

            # ---- GpSimd: base' = bias + g1 - g2 = bias + b2 - b1 ----
            # (runs in parallel with the DVE min/max chain)
            gg = sb.tile([RPC, D], f32)
            nc.gpsimd.tensor_sub(gg[:], b_[:, D:32], b_[:, 0:D])
            base = sb.tile([RPC, D], f32)
            nc.gpsimd.tensor_add(base[:], gg[:], v_bias)

            # ---- DVE: global max / -min of h -> A, B scalars ----
            nc.vector.tensor_reduce(
                scr[0:16, 0:1], v_hT, axis=mybir.AxisListType.X, op=Alu.max
            )
            nc.vector.tensor_reduce(
                scr[0:16, 1:2], v_hT, axis=mybir.AxisListType.X, op=Alu.min,
                negate=True,
            )
            # transpose-fused partition reduces over the NEG-padded block.
            # BIR access patterns cannot start at a nonzero partition, so
            # run the 32x32 transpose-reduce twice with the input window
            # shifted by one column: both results land on partition 0.
            tr = sb.tile([32, 2], f32)
            nc.vector.tensor_reduce(
                tr[:, 0:1], scr[:, 0:32], axis=mybir.AxisListType.X,
                op=Alu.max, apply_transpose=True,
            )
            nc.vector.tensor_reduce(
                tr[:, 1:2], scr[:, 1:33], axis=mybir.AxisListType.X,
                op=Alu.max, apply_transpose=True,
            )
            gmax = tr[0:1, 0:1]   # partition 0, col 0
            mneg = tr[0:1, 1:2]   # partition 0, col 1
            # A = 1 + (r-1)*s, B = (m*r - 0.3)*s with r = 0.6/(g+m).
            # Both fit one (X - C)*s + D with X = [r, m*r], C = [1, 0.3],
            # D = [1, 0] (per-lane consts), halving the tail of the chain.
            sc = sb.tile([1, 8], f32)
            xcol = sb.tile([1, 2], f32)
            nc.vector.tensor_scalar(
                sc[0:1, 2:3], gmax, mneg, None, op0=Alu.max
            )  # tmx
            nc.vector.tensor_scalar(
                sc[0:1, 3:4], sc[0:1, 2:3], 100.0, None, op0=Alu.is_gt
            )  # s
            nc.vector.tensor_scalar(
                sc[0:1, 4:5], gmax, mneg, 1.0 / 0.6, op0=Alu.add, op1=Alu.mult
            )  # rng06 = (gmax + mneg)/0.6
            nc.vector.reciprocal(xcol[0:1, 0:1], sc[0:1, 4:5])  # r
            nc.vector.tensor_scalar(
                xcol[0:1, 1:2], mneg, xcol[0:1, 0:1], None, op0=Alu.mult
            )  # m*r
            z2 = sb.tile([1, 2], f32)
            nc.vector.tensor_sub(z2[0:1, 0:2], xcol[0:1, 0:2], cc[0:1, 0:2])
            nc.vector.scalar_tensor_tensor(
                tb[0:1, 0:2], in0=z2[0:1, 0:2], scalar=sc[0:1, 3:4],
                in1=dd[0:1, 0:2], op0=Alu.mult, op1=Alu.add,
            )  # [A, B]
            bc = sb.tile([32, 2], f32)
            nc.vector.stream_shuffle(bc[:, 0:2], tb[:, 0:2], mask=[0] * 32)

            # ---- combine: logits = A*pm + (B*wsum + base') ----
            xb = sb.tile([RPC, D], f32)
            nc.vector.scalar_tensor_tensor(
                xb[:], in0=ws[:], scalar=bc[0:RPC, 1:2], in1=base[:],
                op0=Alu.mult, op1=Alu.add,
            )
            lg = sb.tile([RPC, D], f32)
            nc.vector.scalar_tensor_tensor(
                lg[:], in0=pm[:], scalar=bc[0:RPC, 0:1], in1=xb[:],
                op0=Alu.mult, op1=Alu.add,
            )

            # ---- top-5 threshold -> hard 0/1 mask ----
            top8 = sb.tile([RPC, 8], f32)
            nc.vector.max(top8[:], lg[:])
            hard = sb.tile([RPC, D], f32)
            nc.vector.tensor_scalar(
                hard[:], lg[:], top8[:, 4:5], None, op0=Alu.is_ge
            )

            i_out = nc.sync.dma_start(out_s[:], hard[:], single_packet=True)

    nc.compile()
    _trim_overhead(nc, mybir, i_out)
    return nc


TRIM_INIT_BARRIER = True
TRIM_EXIT = True
EARLY_OUT_TRIGGER = True
REORDER_SET0_LAST = True


def _trim_overhead(nc, mybir, i_out):
    """Post-compile surgery on the instruction stream.

    The kernel runs in a freshly loaded NEFF (semaphores zeroed at load) and
    is the only tile context, so:
      - the bass init all-engine barrier (between the const-ap memsets and
        user code) protects nothing here; dropping it lets the input-DMA
        triggers issue ~0.9us earlier, right after the NEFF-level preamble;
      - the tile-exit epilogue's double all-engine barrier + semaphore
        range-clear only matter for a following tile context.  Keep just
        SP's wait for the output-DMA completion (which transitively implies
        every upstream op finished) plus its drain, so the NEFF doesn't
        finish with the output DMA in flight;
      - the compiler emits a set-0 ACT table load at block entry in
        addition to the set-5 (Ln) load; no instruction here uses set 0 and
        each load costs ~1.3us of the Activation engine, so drop it;
      - the output-DMA trigger pipeline (DIRECT2D ~0.7us + doorbell-to-
        queue-execute ~0.6us) is far longer than the last two DVE ops
        (top8+is_ge, ~0.45us incl. gaps), so releasing the trigger after
        `logits` instead of after `hard` overlaps the pipeline with the
        compute tail; the queue's SBUF read still lands ~0.9us after
        `hard` is written.
    """
    fn = nc.main_func
    if TRIM_INIT_BARRIER:
        main_b = fn.blocks[0]
        main_b.instructions[:] = [
            i for i in main_b.instructions
            if not isinstance(i, (mybir.InstDrain, mybir.InstEventSemaphore))
        ]
    if REORDER_SET0_LAST:
        # the compiler emits a set-0 table load at ACT stream start ahead
        # of the set-5 (Ln) load; each costs ~1.3us of the Activation
        # engine.  Nothing in this kernel uses set 0, so move its load to
        # the end of the ACT stream where the engine is idle — set 5 then
        # loads ~1.3us earlier and the Gumbel branch follows suit.
        for b in fn.blocks:
            set0 = [
                i for i in b.instructions
                if isinstance(i, mybir.InstLoadActFuncSet)
                and i.act_func_set_id == 0
            ]
            if not set0:
                continue
            assert len(set0) == 1
            b.instructions.remove(set0[0])
            for pos in range(len(b.instructions) - 1, -1, -1):
                inst = b.instructions[pos]
                if (inst.engine == mybir.EngineType.Activation
                        and isinstance(inst, mybir.InstUnconditionalBranch)):
                    b.instructions.insert(pos, set0[0])
                    break
            else:
                b.instructions.append(set0[0])
    out_sems = {u.id for u in (i_out.ins.sync_info.on_update or [])}
    if TRIM_EXIT:
        end_b = fn.blocks[-1]
        kept = []
        for inst in end_b.instructions:
            if isinstance(inst, mybir.InstEventSemaphore):
                si = inst.sync_info
                if si is not None and any(
                    w.id in out_sems for w in (si.on_wait or [])
                ):
                    kept.append(inst)
            elif (isinstance(inst, mybir.InstDrain)
                    and inst.engine == mybir.EngineType.SP):
                kept.append(inst)
                break
        end_b.instructions[:] = kept
    if EARLY_OUT_TRIGGER:
        ws = list(i_out.ins.sync_info.on_wait or [])
        assert len(ws) == 1, ws
        ws[0].wait_value -= 2


def _get_nc():
    if "nc" not in _CACHE:
        _CACHE["nc"] = _build_nc()
    return _CACHE["nc"]


def _make_in_maps(h, w_p, bias, U1, U2):
    h = np.ascontiguousarray(np.asarray(h, np.float32).reshape(ROWS, D))
    hT = h.T
    wp = np.asarray(w_p, np.float32)
    bias = np.asarray(bias, np.float32).reshape(ROWS, D)
    u1 = np.asarray(U1, np.float32).reshape(ROWS, D)
    u2 = np.asarray(U2, np.float32).reshape(ROWS, D)

    in_maps = []
    for c in range(N_CORES):
        rows = slice(c * RPC, (c + 1) * RPC)
        pa = np.empty((16, CS_END), np.float32)
        pa[:, C_HT:C_HTS] = hT
        pa[:, C_HTS:C_ONES] = h[rows].T
        pa[:, C_ONES:C_WP] = 1.0
        pa[:, C_WP:CS_END] = wp
        pb = np.empty((RPC, CA_END), np.float32)
        pb[:, C_U:C_U + D] = u1[rows]
        pb[:, C_U + D:C_BIAS] = u2[rows]
        pb[:, C_BIAS:CA_END] = bias[rows]
        in_maps.append({"packed_sp": pa, "packed_act": pb})
    return in_maps


def kernel(h, input, w_p, bias, U1, U2, **_unused):
    from concourse.bass_utils import run_bass_kernel_spmd

    nc = _get_nc()
    in_maps = _make_in_maps(h, w_p, bias, U1, U2)
    res = run_bass_kernel_spmd(nc, in_maps, core_ids=list(range(N_CORES)))
    out = np.concatenate([r["out_s"] for r in res.results], axis=0)
    return out.reshape(ROWS, 4, 4).astype(np.float32)
